# revision 1
# baseline (speedup 1.0000x reference)
"""Trainium2 Bass kernel for batched cosine-sim bottom-k token mean.

Per example b: sims[l] = <q_b, T_b[l]> / (|q_b| |T_b[l]|); take k=24 smallest,
gather those tokens, mean over them -> [D].

Sharding: pure data-parallel, 32 examples per core x 8 cores.

Per-core algorithm (n_ex examples, T shard flattened [n_ex*576, 1024] f32):
  Phase 1 (streamed per example):
    - DMA tile [128, 5*1024]: partition p, free block j holds token row l=128j+p
      (chunk j=4 only has partitions 0..63 -> rows 512..575).
    - q_b broadcast to [128, 1024] via gpsimd partition_broadcast.
    - DVE tensor_tensor_reduce (mult+add accum) per chunk -> dot[l] column.
    - ACT activation(Square, accum_out) per chunk -> n2[l] column.
  Phase 2 (batched):
    - x = -dot * rsqrt(n2)  (sqrt on ACT, reciprocal+mult on DVE);
      column c = 8b+j layout, pad cols give x=-1e30.
    - PE transpose 128-col blocks -> Y tiles; 32 small SBUF DMAs regroup to
      X [n_ex, 640] (per-example sims contiguous; cols >=576 are -1e30 pads).
    - 3 rounds of max/max_index/match_replace -> 24 smallest indices / example.
  Phase 3:
    - indices -> global row ids; indirect DMA gathers the 24*n_ex rows.
    - PE matmul with 0/1 selection matrix S sums each example's 24 rows;
      ACT copy applies 1/24; DMA out [n_ex, 1024].

The ranking skips |q_b| and the eps clamp (both order-preserving here).
"""

import os
import numpy as np

B, L, D = 256, 576, 1024
KSEL = 24
NCORES = 8
NEG = -1.0e30


def build_nc(n_ex, reps=1, nsplit=2):
    import concourse.bacc as bacc
    import concourse.bass as bass
    import concourse.tile as tile
    import concourse.mybir as mybir

    f32 = mybir.dt.float32
    i32 = mybir.dt.int32
    u32 = mybir.dt.uint32
    Alu = mybir.AluOpType
    Act = mybir.ActivationFunctionType

    rows = n_ex * L
    ncols = 8 * n_ex                      # accumulator columns (8 per example)
    nblk = (ncols + 127) // 128           # 128-col transpose blocks
    gpt = 128 // n_ex                     # candidate slots per gather tile
    ngt = (KSEL + gpt - 1) // gpt         # number of gather tiles

    nc = bacc.Bacc(
        "TRN2",
        target_bir_lowering=False,
        debug=False,
        enable_asserts=False,
        num_devices=1,
    )
    img = nc.dram_tensor("img", [rows, D], f32, kind="ExternalInput")
    qf = nc.dram_tensor("qf", [n_ex, D], f32, kind="ExternalInput")
    offs_d = nc.dram_tensor("offs", [128, nsplit], f32, kind="ExternalInput")
    s_d = nc.dram_tensor("S", [128, n_ex // nsplit], f32, kind="ExternalInput")
    id_d = nc.dram_tensor("ident", [128, 128], f32, kind="ExternalInput")
    out_d = nc.dram_tensor("out", [n_ex, D], f32, kind="ExternalOutput")

    img_ap = img.ap()

    from contextlib import ExitStack

    with tile.TileContext(nc) as tc:
        with ExitStack() as _stk:
            tp = _stk.enter_context(tc.tile_pool(name="tp", bufs=4))
            qp = _stk.enter_context(tc.tile_pool(name="qp", bufs=3))
            sp = _stk.enter_context(tc.tile_pool(name="scratch", bufs=2))
            ap_ = _stk.enter_context(tc.tile_pool(name="acc", bufs=1))
            pp = _stk.enter_context(tc.tile_pool(name="psum", bufs=2, space="PSUM"))
            mp = _stk.enter_context(tc.tile_pool(name="mpsum", bufs=1, space="PSUM"))
            if reps > 1:
                _stk.enter_context(tc.For_i(0, reps, 1))
            # constants
            offs_sb = ap_.tile([128, nsplit], f32, tag="offs")
            nc.sync.dma_start(offs_sb[:], offs_d.ap())
            s_sb = ap_.tile([128, n_ex // nsplit], f32, tag="S")
            nc.sync.dma_start(s_sb[:], s_d.ap())
            id_sb = ap_.tile([128, 128], f32, tag="ident")
            nc.sync.dma_start(id_sb[:], id_d.ap())

            hn = n_ex // nsplit               # examples per split
            hc = 8 * hn                       # accum cols per half
            gpt_h = 128 // hn                 # candidate slots per gather tile
            ngt_h = (KSEL + gpt_h - 1) // gpt_h
            dot_h, n2_h, out_ps = [], [], []
            for h in range(nsplit):
                dh = ap_.tile([128, hc], f32, tag=f"dot{h}", name=f"dot{h}")
                nh2 = ap_.tile([128, hc], f32, tag=f"n2{h}", name=f"n2{h}")
                nc.vector.memset(dh[:], 1.0e30)
                nc.vector.memset(nh2[:], 1.0)
                dot_h.append(dh)
                n2_h.append(nh2)

            def phase23(h):
                nt = ap_.tile([128, hc], f32, tag=f"nt{h}", name=f"nt{h}")
                nc.scalar.sqrt(nt[:], n2_h[h][:])
                inv = ap_.tile([128, hc], f32, tag=f"inv{h}", name=f"inv{h}")
                nc.vector.reciprocal(inv[:], nt[:])
                x_all = ap_.tile([128, hc], f32, tag=f"x{h}", name=f"x{h}")
                nc.vector.scalar_tensor_tensor(
                    out=x_all[:], in0=dot_h[h][:], scalar=-1.0, in1=inv[:],
                    op0=Alu.mult, op1=Alu.mult,
                )
                ys = []
                for blk in range((hc + 127) // 128):
                    w = min(128, hc - 128 * blk)
                    tps = pp.tile([128, 128], f32, tag="tpsum", name="tps")
                    nc.tensor.transpose(
                        tps[0:w, :], x_all[:, 128 * blk : 128 * blk + w], id_sb[:]
                    )
                    y = ap_.tile([128, 128], f32, tag=f"y{h}_{blk}", name=f"y{h}_{blk}")
                    nc.scalar.copy(y[0:w, :], tps[0:w, :])
                    ys.append(y)
                xt = ap_.tile([hn, 640], f32, tag=f"xt{h}", name=f"xt{h}")
                for bl in range(hn):
                    blk, r0 = divmod(8 * bl, 128)
                    nc.sync.dma_start(
                        xt[bl : bl + 1, 0:640], ys[blk][r0 : r0 + 5, 0:128]
                    )
                idxf = ap_.tile([hn, 32], f32, tag=f"idxf{h}", name=f"idxf{h}")
                for r in range(3):
                    mx = ap_.tile([hn, 8], f32, tag=f"mx{h}", name=f"mx{h}")
                    nc.vector.max(mx[:], xt[:])
                    ix = ap_.tile([hn, 8], u32, tag=f"ix{h}", name=f"ix{h}")
                    nc.vector.max_index(ix[:], mx[:], xt[:])
                    nc.vector.match_replace(
                        out=xt[:], in_to_replace=mx[:], in_values=xt[:],
                        imm_value=NEG,
                    )
                    nc.vector.tensor_copy(idxf[:, 8 * r : 8 * r + 8], ix[:])
                idxg = ap_.tile([128, ngt_h], f32, tag=f"idxg{h}", name=f"idxg{h}")
                nc.vector.memset(idxg[:], 0.0)
                for t_i in range(ngt_h):
                    for u in range(gpt_h):
                        m = t_i * gpt_h + u
                        if m >= KSEL:
                            break
                        nc.sync.dma_start(
                            idxg[hn * u : hn * (u + 1), t_i : t_i + 1],
                            idxf[0:hn, m : m + 1],
                        )
                idxg2 = ap_.tile([128, ngt_h], f32, tag=f"idxg2{h}", name=f"idxg2{h}")
                nc.vector.tensor_scalar(
                    out=idxg2[:], in0=idxg[:], scalar1=offs_sb[:, h : h + 1],
                    scalar2=None, op0=Alu.add,
                )
                idxi = ap_.tile([128, ngt_h], i32, tag=f"idxi{h}", name=f"idxi{h}")
                nc.vector.tensor_copy(idxi[:], idxg2[:])

                mean_ps = [
                    mp.tile([hn, 512], f32, tag=f"mps{h}{hh}", name=f"mps{h}{hh}")
                    for hh in range(2)
                ]
                for t_i in range(ngt_h):
                    nrow = min(gpt_h, KSEL - t_i * gpt_h) * hn
                    g = tp.tile([128, D], f32, tag="G", name="G")
                    nc.gpsimd.indirect_dma_start(
                        out=g[0:nrow, :], out_offset=None, in_=img_ap,
                        in_offset=bass.IndirectOffsetOnAxis(
                            ap=idxi[0:nrow, t_i : t_i + 1], axis=0
                        ),
                    )
                    for hh in range(2):
                        nc.tensor.matmul(
                            out=mean_ps[hh][:],
                            lhsT=s_sb[0:nrow, :],
                            rhs=g[0:nrow, 512 * hh : 512 * (hh + 1)],
                            start=(t_i == 0),
                            stop=(t_i == ngt_h - 1),
                        )
                osb = ap_.tile([hn, D], f32, tag=f"osb{h}", name=f"osb{h}")
                for hh in range(2):
                    nc.scalar.mul(
                        osb[:, 512 * hh : 512 * (hh + 1)], mean_ps[hh][:], 1.0 / KSEL
                    )
                nc.sync.dma_start(out_d.ap()[h * hn : (h + 1) * hn, :], osb[:])

            # ---- Phase 1: stream examples; tail per half overlaps next half ----
            for b in range(n_ex):
                h, bl = divmod(b, hn)
                t = tp.tile([128, 5 * 1024], f32, tag="T")
                nc.sync.dma_start(
                    t[:, 0 : 4 * 1024].rearrange("p (j d) -> p j d", j=4),
                    img_ap[L * b : L * b + 512, :].rearrange(
                        "(j p) d -> p j d", p=128
                    ),
                )
                nc.sync.dma_start(
                    t[0:64, 4 * 1024 : 5 * 1024],
                    img_ap[L * b + 512 : L * b + 576, :],
                )
                qrow = qp.tile([1, D], f32, tag="qrow")
                nc.sync.dma_start(qrow[:], qf.ap()[b : b + 1, :])
                qb = qp.tile([128, D], f32, tag="qb")
                nc.gpsimd.partition_broadcast(qb[:], qrow[:])

                for j in range(5):
                    p = 128 if j < 4 else 64
                    chunk = t[0:p, j * 1024 : (j + 1) * 1024]
                    prod = sp.tile([128, D], f32, tag="prod")
                    nc.vector.scalar_tensor_tensor(
                        out=prod[0:p, :],
                        in0=chunk,
                        scalar=1.0,
                        in1=qb[0:p, :],
                        op0=Alu.mult,
                        op1=Alu.mult,
                        accum_out=dot_h[h][0:p, 8 * bl + j : 8 * bl + j + 1],
                    )
                    sq = sp.tile([128, D], f32, tag="sq")
                    nc.scalar.activation(
                        out=sq[0:p, :],
                        in_=chunk,
                        func=Act.Square,
                        accum_out=n2_h[h][0:p, 8 * bl + j : 8 * bl + j + 1],
                    )
                if bl == hn - 1:
                    phase23(h)


    nc.compile()
    return nc


def make_consts(n_ex, nsplit=2):
    hn = n_ex // nsplit
    p = np.arange(128)
    offs = np.stack(
        [(L * (hn * h + p % hn)).astype(np.float32) for h in range(nsplit)], axis=1
    )
    s = (p[:, None] % hn == np.arange(hn)[None, :]).astype(np.float32)
    ident = np.eye(128, dtype=np.float32)
    return {"offs": offs, "S": s, "ident": ident}


_CACHE = {}


NSPLIT = int(os.environ.get("KNN_NSPLIT", "2"))


def _compiled(n_ex):
    key = (n_ex, NSPLIT)
    if key not in _CACHE:
        _CACHE[key] = build_nc(n_ex, nsplit=NSPLIT)
    return _CACHE[key]


def _run_pjrt(nc, in_maps, iters=1):
    """Run the compiled Bass program on NCORES devices via PJRT (axon).

    Mirrors concourse.bass2jax.run_bass_via_pjrt but keeps inputs
    device-resident so repeated executions time the NEFF itself.
    Returns (list-per-core of {name: np.ndarray}, min_exec_seconds).
    """
    import time as _time

    import jax
    import concourse.mybir as mybir
    from concourse import bass2jax
    from jax.sharding import Mesh, NamedSharding, PartitionSpec
    from jax.experimental.shard_map import shard_map

    bass2jax.install_neuronx_cc_hook()

    in_names, out_names, out_avals, zero_outs = [], [], [], []
    for alloc in nc.m.functions[0].allocations:
        if not isinstance(alloc, mybir.MemoryLocationSet):
            continue
        name = alloc.memorylocations[0].name
        if alloc.kind == "ExternalInput":
            in_names.append(name)
        elif alloc.kind == "ExternalOutput":
            out_names.append(name)
            shape = tuple(alloc.tensor_shape)
            dtype = mybir.dt.np(alloc.dtype)
            out_avals.append(jax.core.ShapedArray(shape, dtype))
            zero_outs.append(np.zeros(shape, dtype))
    n_params = len(in_names)
    n_outs = len(out_avals)
    all_names = in_names + out_names

    def _body(*args):
        outs = bass2jax._bass_exec_p.bind(
            *args,
            out_avals=tuple(out_avals),
            in_names=tuple(all_names),
            out_names=tuple(out_names),
            lowering_input_output_aliases=(),
            sim_require_finite=True,
            sim_require_nnan=True,
            nc=nc,
        )
        return tuple(outs)

    n_cores = len(in_maps)
    devices = jax.devices()[:n_cores]
    mesh = Mesh(np.asarray(devices), ("core",))
    spec = PartitionSpec("core")
    sharding = NamedSharding(mesh, spec)
    donate = tuple(range(n_params, n_params + n_outs))
    sharded = jax.jit(
        shard_map(
            _body,
            mesh=mesh,
            in_specs=(spec,) * (n_params + n_outs),
            out_specs=(spec,) * n_outs,
            check_rep=False,
        ),
        donate_argnums=donate,
        keep_unused=True,
    )
    pid_name = nc.partition_id_tensor.name if nc.partition_id_tensor else None
    name_avals = {}
    for alloc in nc.m.functions[0].allocations:
        if isinstance(alloc, mybir.MemoryLocationSet) and alloc.kind == "ExternalInput":
            name_avals[alloc.memorylocations[0].name] = (
                tuple(alloc.tensor_shape),
                mybir.dt.np(alloc.dtype),
            )

    def core_input(m, name, c):
        if name == pid_name:
            shape, dtype = name_avals[name]
            return np.full(shape, c, dtype=dtype)
        return np.asarray(m[name])

    concat_in = [
        np.concatenate(
            [core_input(m, name, c) for c, m in enumerate(in_maps)], axis=0
        )
        for name in in_names
    ]
    dev_in = [jax.device_put(a, sharding) for a in concat_in]
    jax.block_until_ready(dev_in)

    best = None
    out_arrs = None
    for _ in range(max(1, iters)):
        zeros = [
            jax.device_put(np.zeros((n_cores * z.shape[0], *z.shape[1:]), z.dtype), sharding)
            for z in zero_outs
        ]
        jax.block_until_ready(zeros)
        t0 = _time.perf_counter()
        out_arrs = sharded(*dev_in, *zeros)
        jax.block_until_ready(out_arrs)
        dt = _time.perf_counter() - t0
        best = dt if best is None else min(best, dt)

    results = [
        {
            name: np.asarray(out_arrs[i]).reshape(n_cores, *out_avals[i].shape)[c]
            for i, name in enumerate(out_names)
        }
        for c in range(n_cores)
    ]
    return results, best


def kernel(i_feats, image_feats, k):
    assert int(k) == KSEL
    i_feats = np.ascontiguousarray(np.asarray(i_feats), dtype=np.float32)
    image_feats = np.ascontiguousarray(np.asarray(image_feats), dtype=np.float32)
    assert i_feats.shape == (B, D) and image_feats.shape == (B, L, D)
    n_ex = B // NCORES

    nc = _compiled(n_ex)
    consts = make_consts(n_ex, NSPLIT)
    in_maps = []
    for c in range(NCORES):
        sl = slice(n_ex * c, n_ex * (c + 1))
        in_maps.append(
            {
                "img": image_feats[sl].reshape(n_ex * L, D),
                "qf": i_feats[sl],
                **consts,
            }
        )

    iters = int(os.environ.get("KNN_TIME_ITERS", "1"))
    results, best = _run_pjrt(nc, in_maps, iters=iters)
    kernel.exec_time_s = best
    out = np.concatenate([results[c]["out"] for c in range(NCORES)], axis=0)
    return out

